# revision 16
# baseline (speedup 1.0000x reference)
"""KNN classification kernel for Trainium2 (Bass/Tile), 8-core SPMD.

Problem: 1-query KNN over train_data [500000, 256] f32, K=3, 10 classes.
    distances = ||x - train_data||_2  -> top-3 smallest -> mode of targets.

Strategy (fp8 TensorE scan + exact host refinement):
  - Rank by m(t) = 2<x,t> - ||t||^2 (== -d^2 up to the constant ||x||^2).
  - Host precomputes exact fp32 row norms ||t||^2 and ships train_data
    TRANSPOSED in fp8 E4M3 (4x less HBM traffic than f32: 16MB/core).
  - Each core's TensorE computes <x,t> for its 62500 rows: the data tile
    is the STATIONARY operand (lhsT [128 dims x 128 rows], FWL-accelerated
    fp8 weight loads) and x is a 1-column moving operand; each 128-row
    block accumulates a [128,1] psum column over the two 128-dim chunks.
  - DVE epilogue (per column-half, so half A overlaps the matmul stream):
    m = 2*psum - norms, then max_with_indices -> per-partition top-8
    (value, block) candidates; DMA'd out.
  - Host merges 8 cores x 128 partitions x 16 candidates, recomputes
    EXACT fp64 distances for them from the original f32 data, takes the
    global top-3 by (distance, index) and the mode with smallest-on-tie.
    fp8 ranking error (std ~1.1) vs candidate margins (~70) makes the
    top-3 containment rock-solid; the refinement makes the result exact.

Memory-bound target: per-core 16MB fp8 / ~400 GB/s ~= 40us stream; one
DMA per tile (both dim-chunks in a 3D AP) alternating across the two
HWDGE issue queues; TensorE ~27us of FWL weight loads + FD=1 matmuls
hides under the DMA stream.
"""

import sys

import numpy as np

for _p in ("/opt/trn_rl_repo",):
    if _p not in sys.path:
        sys.path.insert(0, _p)

import ml_dtypes

import concourse.bacc as bacc
import concourse.mybir as mybir
from concourse import tile
from concourse.bass_utils import run_bass_kernel_spmd

N_TRAIN = 500000
D = 256
CORES = 8
K = 3
P = 128
NS = N_TRAIN // CORES  # 62500 rows per core
BLOCKS = -(-NS // P)  # 489 psum columns
NSP = BLOCKS * P  # 62592 padded rows per core
BIG = 1.0e30
FP32 = mybir.dt.float32
F8 = mybir.dt.float8e4
U32 = mybir.dt.uint32
NP_F8 = ml_dtypes.float8_e4m3
SPLIT = 256  # epilogue half boundary (psum bank = 512 f32 max)


def build_knn(tc):
    """Per-core program: fp8 dot products via TensorE + top-8 epilogue."""
    nc = tc.nc
    x_ap = nc.dram_tensor("xq", [P, 2], F8, kind="ExternalInput").ap()
    a_ap = nc.dram_tensor("a", [2 * P, NSP], F8, kind="ExternalInput").ap()
    nrm_ap = nc.dram_tensor("nrm", [P, BLOCKS], FP32, kind="ExternalInput").ap()
    va_ap = nc.dram_tensor("out_vals_a", [P, 8], FP32, kind="ExternalOutput").ap()
    ia_ap = nc.dram_tensor("out_idx_a", [P, 8], U32, kind="ExternalOutput").ap()
    vb_ap = nc.dram_tensor("out_vals_b", [P, 8], FP32, kind="ExternalOutput").ap()
    ib_ap = nc.dram_tensor("out_idx_b", [P, 8], U32, kind="ExternalOutput").ap()

    with (
        tc.tile_pool(name="xp", bufs=1) as x_pool,
        tc.tile_pool(name="np", bufs=1) as n_pool,
        tc.tile_pool(name="inp", bufs=1) as in_pool,
        tc.tile_pool(name="psp", bufs=1, space="PSUM") as ps_pool,
        tc.tile_pool(name="outp", bufs=1) as out_pool,
    ):
        # norms ride along early so they can't straggle into the epilogue
        nrm_t = n_pool.tile([P, BLOCKS], FP32)
        nc.scalar.dma_start(out=nrm_t[:], in_=nrm_ap[:, :])
        x_t = x_pool.tile([P, 2], F8)
        nc.sync.dma_start(out=x_t[:], in_=x_ap[:, :])

        # Two psum tiles so half A's epilogue overlaps half B's matmuls.
        ps_a = ps_pool.tile([P, SPLIT], FP32)
        ps_b = ps_pool.tile([P, BLOCKS - SPLIT], FP32)

        # One DMA per tile (both 128-dim chunks via 3D AP), cycling across
        # the SP/ACT HWDGE queues and the GpSimd SWDGE queue (extra
        # completion-tracking lanes). Small tiles first: the matmuls start
        # early AND the 8-lane reuse conflicts always land on transfers
        # that finished long ago. Tiny tail to shorten the last drain.
        blocks_per_tile = [4, 6, 8, 10, 12, 14, 16, 20, 24, 28, 32, 36,
                           40, 40, 40, 40, 40, 36, 24, 12, 5, 2]
        assert sum(blocks_per_tile) == BLOCKS

        def epilogue_half(tag, lo, hi, ps, v_ap, i_ap):
            w = hi - lo
            m_t = out_pool.tile([P, w], FP32, tag=f"m_{tag}", name=f"m_{tag}")
            nc.vector.scalar_tensor_tensor(
                out=m_t[:],
                in0=ps[:],
                scalar=2.0,
                in1=nrm_t[:, lo:hi],
                op0=mybir.AluOpType.mult,
                op1=mybir.AluOpType.subtract,
            )
            valt = out_pool.tile([P, 8], FP32, tag=f"v_{tag}", name=f"v_{tag}")
            idxt = out_pool.tile([P, 8], U32, tag=f"i_{tag}", name=f"i_{tag}")
            nc.vector.max_with_indices(valt[:], idxt[:], m_t[:])
            nc.sync.dma_start(out=v_ap[:, :], in_=valt[:])
            nc.sync.dma_start(out=i_ap[:, :], in_=idxt[:])

        col = 0
        r = 0
        for ti, nb in enumerate(blocks_per_tile):
            rt = nb * P
            t = in_pool.tile([P, 2, rt], F8, tag=f"t_{ti}", name=f"t_{ti}")
            eng = (nc.sync, nc.scalar, nc.gpsimd)[ti % 3]
            eng.dma_start(
                out=t[:, :, 0:rt],
                in_=a_ap[:, r : r + rt].rearrange("(o k) r -> k o r", k=P),
            )
            for b in range(rt // P):
                if col < SPLIT:
                    pcol, ps = col, ps_a
                else:
                    pcol, ps = col - SPLIT, ps_b
                nc.tensor.matmul(
                    ps[:, pcol : pcol + 1],
                    lhsT=t[:, 0, b * P : (b + 1) * P],
                    rhs=x_t[:, 0:1],
                    start=True,
                    stop=False,
                )
                nc.tensor.matmul(
                    ps[:, pcol : pcol + 1],
                    lhsT=t[:, 1, b * P : (b + 1) * P],
                    rhs=x_t[:, 1:2],
                    start=False,
                    stop=True,
                )
                col += 1
                if col == SPLIT:
                    epilogue_half("a", 0, SPLIT, ps_a, va_ap, ia_ap)
            r += rt
        assert col == BLOCKS and r == NSP

        epilogue_half("b", SPLIT, BLOCKS, ps_b, vb_ap, ib_ap)


_PROGRAM_CACHE = {}


def get_program():
    if "nc" not in _PROGRAM_CACHE:
        nc = bacc.Bacc(
            "TRN2", target_bir_lowering=False, debug=False, num_devices=CORES
        )
        with tile.TileContext(nc) as tc:
            build_knn(tc)
        nc.compile()
        _PROGRAM_CACHE["nc"] = nc
    return _PROGRAM_CACHE["nc"]


def run_device(in_maps, trace=False, trace_cores=None):
    nc = get_program()
    return run_bass_kernel_spmd(
        nc, in_maps, list(range(CORES)), trace=trace, trace_cores=trace_cores
    )


def make_in_maps(x, train_data):
    x = np.asarray(x, dtype=np.float32)
    train_data = np.asarray(train_data, dtype=np.float32)
    td8 = train_data.astype(NP_F8)
    x8 = x.astype(NP_F8)
    xq = np.ascontiguousarray(x8.reshape(2, P).T)  # [128, 2]
    norms = np.einsum("nd,nd->n", train_data, train_data, dtype=np.float64)
    norms = norms.astype(np.float32)
    in_maps = []
    for c in range(CORES):
        a = np.zeros((2 * P, NSP), dtype=NP_F8)
        a[:, :NS] = td8[c * NS : (c + 1) * NS].T
        nrm = np.full(NSP, BIG, dtype=np.float32)
        nrm[:NS] = norms[c * NS : (c + 1) * NS]
        nrm = np.ascontiguousarray(nrm.reshape(BLOCKS, P).T)  # [128, BLOCKS]
        in_maps.append({"xq": xq, "a": a, "nrm": nrm})
    return in_maps


def merge_results(results, x, train_data, train_targets):
    """Merge per-core candidates; refine with exact distances on host."""
    x64 = np.asarray(x, dtype=np.float64)
    td = np.asarray(train_data)
    p_idx = np.arange(P, dtype=np.int64)[:, None]
    cands = []
    for c in range(len(results)):
        for vk, ik, off in (
            ("out_vals_a", "out_idx_a", 0),
            ("out_vals_b", "out_idx_b", SPLIT),
        ):
            v = np.asarray(results[c][vk], dtype=np.float64)
            ix = np.asarray(results[c][ik], dtype=np.int64) + off
            rl = ix * P + p_idx  # core-local row
            valid = (v > -BIG / 2) & (rl < NS)
            cands.append((c * NS + rl)[valid])
    g = np.unique(np.concatenate(cands))
    d2 = ((td[g].astype(np.float64) - x64) ** 2).sum(axis=1)
    order = np.lexsort((g, d2))  # distance asc, then index asc (top_k ties)
    top = g[order[:K]]
    knn_t = np.asarray(train_targets)[top]
    # torch .mode(): most frequent value, smallest value on ties
    counts = (knn_t[:, None] == knn_t[None, :]).sum(axis=1)
    sentinel = np.iinfo(knn_t.dtype).max
    cands_cls = np.where(counts == counts.max(), knn_t, sentinel)
    return cands_cls.min()


def kernel(x, train_data, train_targets):
    train_targets = np.asarray(train_targets)
    in_maps = make_in_maps(x, train_data)
    results = run_device(in_maps).results
    pred = merge_results(results, x, train_data, train_targets)
    return np.array(pred, dtype=train_targets.dtype)


# revision 18
# speedup vs baseline: 1.0346x; 1.0346x over previous
"""KNN classification kernel for Trainium2 (Bass/Tile), 8-core SPMD.

Problem: 1-query KNN over train_data [500000, 256] f32, K=3, 10 classes.
    distances = ||x - train_data||_2  -> top-3 smallest -> mode of targets.

Strategy (fp8 TensorE scan + exact host refinement):
  - Rank by m(t) = 2<x,t> - ||t||^2 (== -d^2 up to the constant ||x||^2).
  - Host precomputes exact fp32 row norms ||t||^2 and ships train_data
    TRANSPOSED in fp8 E4M3 (4x less HBM traffic than f32: 16MB/core).
  - Each core's TensorE computes <x,t> for its 62500 rows: the data tile
    is the STATIONARY operand (lhsT [128 dims x 128 rows], FWL-accelerated
    fp8 weight loads) and x is a 1-column moving operand; each 128-row
    block accumulates a [128,1] psum column over the two 128-dim chunks.
  - DVE epilogue (per column-half, so half A overlaps the matmul stream):
    m = 2*psum - norms, then max_with_indices -> per-partition top-8
    (value, block) candidates; DMA'd out.
  - Host merges 8 cores x 128 partitions x 16 candidates, recomputes
    EXACT fp64 distances for them from the original f32 data, takes the
    global top-3 by (distance, index) and the mode with smallest-on-tie.
    fp8 ranking error (std ~1.1) vs candidate margins (~70) makes the
    top-3 containment rock-solid; the refinement makes the result exact.

Memory-bound target: per-core 16MB fp8 / ~400 GB/s ~= 40us stream; one
DMA per tile (both dim-chunks in a 3D AP) alternating across the two
HWDGE issue queues; TensorE ~27us of FWL weight loads + FD=1 matmuls
hides under the DMA stream.
"""

import sys

import numpy as np

for _p in ("/opt/trn_rl_repo",):
    if _p not in sys.path:
        sys.path.insert(0, _p)

import ml_dtypes

import concourse.bacc as bacc
import concourse.mybir as mybir
from concourse import tile
from concourse.bass_utils import run_bass_kernel_spmd

N_TRAIN = 500000
D = 256
CORES = 8
K = 3
P = 128
NS = N_TRAIN // CORES  # 62500 rows per core
BLOCKS = -(-NS // P)  # 489 psum columns
NSP = BLOCKS * P  # 62592 padded rows per core
BIG = 1.0e30
FP32 = mybir.dt.float32
F8 = mybir.dt.float8e4
U32 = mybir.dt.uint32
NP_F8 = ml_dtypes.float8_e4m3
SPLIT = 256  # epilogue half boundary (psum bank = 512 f32 max)


def build_knn(tc):
    """Per-core program: fp8 dot products via TensorE + top-8 epilogue."""
    nc = tc.nc
    x_ap = nc.dram_tensor("xq", [P, 2], F8, kind="ExternalInput").ap()
    a_ap = nc.dram_tensor("a", [2 * P, NSP], F8, kind="ExternalInput").ap()
    nrm_ap = nc.dram_tensor("nrm", [P, BLOCKS], FP32, kind="ExternalInput").ap()
    va_ap = nc.dram_tensor("out_vals_a", [P, 8], FP32, kind="ExternalOutput").ap()
    ia_ap = nc.dram_tensor("out_idx_a", [P, 8], U32, kind="ExternalOutput").ap()
    vb_ap = nc.dram_tensor("out_vals_b", [P, 8], FP32, kind="ExternalOutput").ap()
    ib_ap = nc.dram_tensor("out_idx_b", [P, 8], U32, kind="ExternalOutput").ap()

    with (
        tc.tile_pool(name="xp", bufs=1) as x_pool,
        tc.tile_pool(name="np", bufs=1) as n_pool,
        tc.tile_pool(name="inp", bufs=1) as in_pool,
        tc.tile_pool(name="psp", bufs=1, space="PSUM") as ps_pool,
        tc.tile_pool(name="outp", bufs=1) as out_pool,
    ):
        # norms ride along early so they can't straggle into the epilogue
        nrm_t = n_pool.tile([P, BLOCKS], FP32)
        nc.scalar.dma_start(out=nrm_t[:], in_=nrm_ap[:, :])
        x_t = x_pool.tile([P, 2], F8)
        nc.sync.dma_start(out=x_t[:], in_=x_ap[:, :])

        # Two psum tiles so half A's epilogue overlaps half B's matmuls.
        ps_a = ps_pool.tile([P, SPLIT], FP32)
        ps_b = ps_pool.tile([P, BLOCKS - SPLIT], FP32)

        # One DMA per tile (both 128-dim chunks via 3D AP) on the SP HWDGE
        # queue; norms ride the ACT queue so they can't straggle. Uniform
        # big tiles, tapered tail.
        blocks_per_tile = [64] * 7 + [32, 9]
        assert sum(blocks_per_tile) == BLOCKS

        def epilogue_half(tag, lo, hi, ps, v_ap, i_ap):
            w = hi - lo
            m_t = out_pool.tile([P, w], FP32, tag=f"m_{tag}", name=f"m_{tag}")
            nc.vector.scalar_tensor_tensor(
                out=m_t[:],
                in0=ps[:],
                scalar=2.0,
                in1=nrm_t[:, lo:hi],
                op0=mybir.AluOpType.mult,
                op1=mybir.AluOpType.subtract,
            )
            valt = out_pool.tile([P, 8], FP32, tag=f"v_{tag}", name=f"v_{tag}")
            idxt = out_pool.tile([P, 8], U32, tag=f"i_{tag}", name=f"i_{tag}")
            nc.vector.max_with_indices(valt[:], idxt[:], m_t[:])
            nc.sync.dma_start(out=v_ap[:, :], in_=valt[:])
            nc.sync.dma_start(out=i_ap[:, :], in_=idxt[:])

        col = 0
        r = 0
        for ti, nb in enumerate(blocks_per_tile):
            rt = nb * P
            t = in_pool.tile([P, 2, rt], F8, tag=f"t_{ti}", name=f"t_{ti}")
            eng = nc.sync
            eng.dma_start(
                out=t[:, :, 0:rt],
                in_=a_ap[:, r : r + rt].rearrange("(o k) r -> k o r", k=P),
            )
            for b in range(rt // P):
                if col < SPLIT:
                    pcol, ps = col, ps_a
                else:
                    pcol, ps = col - SPLIT, ps_b
                nc.tensor.matmul(
                    ps[:, pcol : pcol + 1],
                    lhsT=t[:, 0, b * P : (b + 1) * P],
                    rhs=x_t[:, 0:1],
                    start=True,
                    stop=False,
                )
                nc.tensor.matmul(
                    ps[:, pcol : pcol + 1],
                    lhsT=t[:, 1, b * P : (b + 1) * P],
                    rhs=x_t[:, 1:2],
                    start=False,
                    stop=True,
                )
                col += 1
                if col == SPLIT:
                    epilogue_half("a", 0, SPLIT, ps_a, va_ap, ia_ap)
            r += rt
        assert col == BLOCKS and r == NSP

        epilogue_half("b", SPLIT, BLOCKS, ps_b, vb_ap, ib_ap)


_PROGRAM_CACHE = {}


def get_program():
    if "nc" not in _PROGRAM_CACHE:
        nc = bacc.Bacc(
            "TRN2", target_bir_lowering=False, debug=False, num_devices=CORES
        )
        with tile.TileContext(nc) as tc:
            build_knn(tc)
        nc.compile()
        _PROGRAM_CACHE["nc"] = nc
    return _PROGRAM_CACHE["nc"]


def run_device(in_maps, trace=False, trace_cores=None):
    nc = get_program()
    return run_bass_kernel_spmd(
        nc, in_maps, list(range(CORES)), trace=trace, trace_cores=trace_cores
    )


def make_in_maps(x, train_data):
    x = np.asarray(x, dtype=np.float32)
    train_data = np.asarray(train_data, dtype=np.float32)
    td8 = train_data.astype(NP_F8)
    x8 = x.astype(NP_F8)
    xq = np.ascontiguousarray(x8.reshape(2, P).T)  # [128, 2]
    norms = np.einsum("nd,nd->n", train_data, train_data, dtype=np.float64)
    norms = norms.astype(np.float32)
    in_maps = []
    for c in range(CORES):
        a = np.zeros((2 * P, NSP), dtype=NP_F8)
        a[:, :NS] = td8[c * NS : (c + 1) * NS].T
        nrm = np.full(NSP, BIG, dtype=np.float32)
        nrm[:NS] = norms[c * NS : (c + 1) * NS]
        nrm = np.ascontiguousarray(nrm.reshape(BLOCKS, P).T)  # [128, BLOCKS]
        in_maps.append({"xq": xq, "a": a, "nrm": nrm})
    return in_maps


def merge_results(results, x, train_data, train_targets):
    """Merge per-core candidates; refine with exact distances on host."""
    x64 = np.asarray(x, dtype=np.float64)
    td = np.asarray(train_data)
    p_idx = np.arange(P, dtype=np.int64)[:, None]
    cands = []
    for c in range(len(results)):
        for vk, ik, off in (
            ("out_vals_a", "out_idx_a", 0),
            ("out_vals_b", "out_idx_b", SPLIT),
        ):
            v = np.asarray(results[c][vk], dtype=np.float64)
            ix = np.asarray(results[c][ik], dtype=np.int64) + off
            rl = ix * P + p_idx  # core-local row
            valid = (v > -BIG / 2) & (rl < NS)
            cands.append((c * NS + rl)[valid])
    g = np.unique(np.concatenate(cands))
    d2 = ((td[g].astype(np.float64) - x64) ** 2).sum(axis=1)
    order = np.lexsort((g, d2))  # distance asc, then index asc (top_k ties)
    top = g[order[:K]]
    knn_t = np.asarray(train_targets)[top]
    # torch .mode(): most frequent value, smallest value on ties
    counts = (knn_t[:, None] == knn_t[None, :]).sum(axis=1)
    sentinel = np.iinfo(knn_t.dtype).max
    cands_cls = np.where(counts == counts.max(), knn_t, sentinel)
    return cands_cls.min()


def kernel(x, train_data, train_targets):
    train_targets = np.asarray(train_targets)
    in_maps = make_in_maps(x, train_data)
    results = run_device(in_maps).results
    pred = merge_results(results, x, train_data, train_targets)
    return np.array(pred, dtype=train_targets.dtype)


# revision 20
# speedup vs baseline: 1.0998x; 1.0630x over previous
"""KNN classification kernel for Trainium2 (Bass/Tile), 8-core SPMD.

Problem: 1-query KNN over train_data [500000, 256] f32, K=3, 10 classes.
    distances = ||x - train_data||_2  -> top-3 smallest -> mode of targets.

Strategy (fp8 TensorE scan + exact host refinement):
  - Rank by m(t) = 2<x,t> - ||t||^2 (== -d^2 up to the constant ||x||^2).
  - Host precomputes exact fp32 row norms ||t||^2 and ships train_data
    TRANSPOSED in fp8 E4M3 (4x less HBM traffic than f32: 16MB/core).
  - Each core's TensorE computes <x,t> for its 62500 rows: the data tile
    is the STATIONARY operand (lhsT [128 dims x 128 rows], FWL-accelerated
    fp8 weight loads) and x is a 1-column moving operand; each 128-row
    block accumulates a [128,1] psum column over the two 128-dim chunks.
  - DVE epilogue (per column-half, so half A overlaps the matmul stream):
    m = 2*psum - norms, then max_with_indices -> per-partition top-8
    (value, block) candidates; DMA'd out.
  - Host merges 8 cores x 128 partitions x 16 candidates, recomputes
    EXACT fp64 distances for them from the original f32 data, takes the
    global top-3 by (distance, index) and the mode with smallest-on-tie.
    fp8 ranking error (std ~1.1) vs candidate margins (~70) makes the
    top-3 containment rock-solid; the refinement makes the result exact.

Memory-bound target: per-core 16MB fp8 / ~400 GB/s ~= 40us stream; one
DMA per tile (both dim-chunks in a 3D AP) alternating across the two
HWDGE issue queues; TensorE ~27us of FWL weight loads + FD=1 matmuls
hides under the DMA stream.
"""

import sys

import numpy as np

for _p in ("/opt/trn_rl_repo",):
    if _p not in sys.path:
        sys.path.insert(0, _p)

import ml_dtypes

import concourse.bacc as bacc
import concourse.mybir as mybir
from concourse import tile
from concourse.bass_utils import run_bass_kernel_spmd

N_TRAIN = 500000
D = 256
CORES = 8
K = 3
P = 128
NS = N_TRAIN // CORES  # 62500 rows per core
BLOCKS = -(-NS // P)  # 489 psum columns
NSP = BLOCKS * P  # 62592 padded rows per core
BIG = 1.0e30
FP32 = mybir.dt.float32
F8 = mybir.dt.float8e4
U32 = mybir.dt.uint32
NP_F8 = ml_dtypes.float8_e4m3
SPLIT = 256  # epilogue half boundary (psum bank = 512 f32 max)


def build_knn(tc):
    """Per-core program: fp8 dot products via TensorE + top-8 epilogue."""
    nc = tc.nc
    x_ap = nc.dram_tensor("xq", [P, 2], F8, kind="ExternalInput").ap()
    a_ap = nc.dram_tensor("a", [2 * P, NSP], F8, kind="ExternalInput").ap()
    nrm_ap = nc.dram_tensor("nrm", [P, BLOCKS], FP32, kind="ExternalInput").ap()
    va_ap = nc.dram_tensor("out_vals_a", [P, 8], FP32, kind="ExternalOutput").ap()
    ia_ap = nc.dram_tensor("out_idx_a", [P, 8], U32, kind="ExternalOutput").ap()
    vb_ap = nc.dram_tensor("out_vals_b", [P, 8], FP32, kind="ExternalOutput").ap()
    ib_ap = nc.dram_tensor("out_idx_b", [P, 8], U32, kind="ExternalOutput").ap()

    with (
        tc.tile_pool(name="xp", bufs=1) as x_pool,
        tc.tile_pool(name="np", bufs=1) as n_pool,
        tc.tile_pool(name="inp", bufs=3) as in_pool,
        tc.tile_pool(name="psp", bufs=1, space="PSUM") as ps_pool,
        tc.tile_pool(name="outp", bufs=1) as out_pool,
    ):
        x_t = x_pool.tile([P, 2], F8)
        nc.sync.dma_start(out=x_t[:], in_=x_ap[:, :])
        nrm_t = n_pool.tile([P, BLOCKS], FP32)
        nc.sync.dma_start(out=nrm_t[:], in_=nrm_ap[:, :])

        # Two psum tiles so half A's epilogue overlaps half B's matmuls.
        ps_a = ps_pool.tile([P, SPLIT], FP32)
        ps_b = ps_pool.tile([P, BLOCKS - SPLIT], FP32)

        RT = 8192

        def epilogue_half(tag, lo, hi, ps, v_ap, i_ap):
            w = hi - lo
            m_t = out_pool.tile([P, w], FP32, tag=f"m_{tag}", name=f"m_{tag}")
            nc.vector.scalar_tensor_tensor(
                out=m_t[:],
                in0=ps[:],
                scalar=2.0,
                in1=nrm_t[:, lo:hi],
                op0=mybir.AluOpType.mult,
                op1=mybir.AluOpType.subtract,
            )
            valt = out_pool.tile([P, 8], FP32, tag=f"v_{tag}", name=f"v_{tag}")
            idxt = out_pool.tile([P, 8], U32, tag=f"i_{tag}", name=f"i_{tag}")
            nc.vector.max_with_indices(valt[:], idxt[:], m_t[:])
            nc.sync.dma_start(out=v_ap[:, :], in_=valt[:])
            nc.sync.dma_start(out=i_ap[:, :], in_=idxt[:])

        col = 0
        r = 0
        while r < NSP:
            rt = min(RT, NSP - r)
            t0 = in_pool.tile([P, RT], F8, tag="c0")
            t1 = in_pool.tile([P, RT], F8, tag="c1")
            nc.sync.dma_start(out=t0[:, 0:rt], in_=a_ap[0:P, r : r + rt])
            nc.sync.dma_start(out=t1[:, 0:rt], in_=a_ap[P : 2 * P, r : r + rt])
            for b in range(rt // P):
                if col < SPLIT:
                    pcol, ps = col, ps_a
                else:
                    pcol, ps = col - SPLIT, ps_b
                nc.tensor.matmul(
                    ps[:, pcol : pcol + 1],
                    lhsT=t0[:, b * P : (b + 1) * P],
                    rhs=x_t[:, 0:1],
                    start=True,
                    stop=False,
                )
                nc.tensor.matmul(
                    ps[:, pcol : pcol + 1],
                    lhsT=t1[:, b * P : (b + 1) * P],
                    rhs=x_t[:, 1:2],
                    start=False,
                    stop=True,
                )
                col += 1
                if col == SPLIT:
                    epilogue_half("a", 0, SPLIT, ps_a, va_ap, ia_ap)
            r += rt
        assert col == BLOCKS and r == NSP

        epilogue_half("b", SPLIT, BLOCKS, ps_b, vb_ap, ib_ap)


_PROGRAM_CACHE = {}


def get_program():
    if "nc" not in _PROGRAM_CACHE:
        nc = bacc.Bacc(
            "TRN2", target_bir_lowering=False, debug=False, num_devices=CORES
        )
        with tile.TileContext(nc) as tc:
            build_knn(tc)
        nc.compile()
        _PROGRAM_CACHE["nc"] = nc
    return _PROGRAM_CACHE["nc"]


def run_device(in_maps, trace=False, trace_cores=None):
    nc = get_program()
    return run_bass_kernel_spmd(
        nc, in_maps, list(range(CORES)), trace=trace, trace_cores=trace_cores
    )


def make_in_maps(x, train_data):
    x = np.asarray(x, dtype=np.float32)
    train_data = np.asarray(train_data, dtype=np.float32)
    td8 = train_data.astype(NP_F8)
    x8 = x.astype(NP_F8)
    xq = np.ascontiguousarray(x8.reshape(2, P).T)  # [128, 2]
    norms = np.einsum("nd,nd->n", train_data, train_data, dtype=np.float64)
    norms = norms.astype(np.float32)
    in_maps = []
    for c in range(CORES):
        a = np.zeros((2 * P, NSP), dtype=NP_F8)
        a[:, :NS] = td8[c * NS : (c + 1) * NS].T
        nrm = np.full(NSP, BIG, dtype=np.float32)
        nrm[:NS] = norms[c * NS : (c + 1) * NS]
        nrm = np.ascontiguousarray(nrm.reshape(BLOCKS, P).T)  # [128, BLOCKS]
        in_maps.append({"xq": xq, "a": a, "nrm": nrm})
    return in_maps


def merge_results(results, x, train_data, train_targets):
    """Merge per-core candidates; refine with exact distances on host."""
    x64 = np.asarray(x, dtype=np.float64)
    td = np.asarray(train_data)
    p_idx = np.arange(P, dtype=np.int64)[:, None]
    cands = []
    for c in range(len(results)):
        for vk, ik, off in (
            ("out_vals_a", "out_idx_a", 0),
            ("out_vals_b", "out_idx_b", SPLIT),
        ):
            v = np.asarray(results[c][vk], dtype=np.float64)
            ix = np.asarray(results[c][ik], dtype=np.int64) + off
            rl = ix * P + p_idx  # core-local row
            valid = (v > -BIG / 2) & (rl < NS)
            cands.append((c * NS + rl)[valid])
    g = np.unique(np.concatenate(cands))
    d2 = ((td[g].astype(np.float64) - x64) ** 2).sum(axis=1)
    order = np.lexsort((g, d2))  # distance asc, then index asc (top_k ties)
    top = g[order[:K]]
    knn_t = np.asarray(train_targets)[top]
    # torch .mode(): most frequent value, smallest value on ties
    counts = (knn_t[:, None] == knn_t[None, :]).sum(axis=1)
    sentinel = np.iinfo(knn_t.dtype).max
    cands_cls = np.where(counts == counts.max(), knn_t, sentinel)
    return cands_cls.min()


def kernel(x, train_data, train_targets):
    train_targets = np.asarray(train_targets)
    in_maps = make_in_maps(x, train_data)
    results = run_device(in_maps).results
    pred = merge_results(results, x, train_data, train_targets)
    return np.array(pred, dtype=train_targets.dtype)


# revision 22
# speedup vs baseline: 1.1772x; 1.0704x over previous
"""KNN classification kernel for Trainium2 (Bass/Tile), 8-core SPMD.

Problem: 1-query KNN over train_data [500000, 256] f32, K=3, 10 classes.
    distances = ||x - train_data||_2  -> top-3 smallest -> mode of targets.

Strategy (fp8 TensorE scan + exact host refinement):
  - Rank by m(t) = 2<x,t> - ||t||^2 (== -d^2 up to the constant ||x||^2).
  - Host precomputes exact fp32 row norms ||t||^2 and ships train_data
    TRANSPOSED in fp8 E4M3 (4x less HBM traffic than f32: 16MB/core).
  - Each core's TensorE computes <x,t> for its 62500 rows: the data tile
    is the STATIONARY operand (lhsT [128 dims x 128 rows], FWL-accelerated
    fp8 weight loads) and x is a 1-column moving operand; each 128-row
    block accumulates a [128,1] psum column over the two 128-dim chunks.
  - DVE epilogue (per column-half, so half A overlaps the matmul stream):
    m = 2*psum - norms, then max_with_indices -> per-partition top-8
    (value, block) candidates; DMA'd out.
  - Host merges 8 cores x 128 partitions x 16 candidates, recomputes
    EXACT fp64 distances for them from the original f32 data, takes the
    global top-3 by (distance, index) and the mode with smallest-on-tie.
    fp8 ranking error (std ~1.1) vs candidate margins (~70) makes the
    top-3 containment rock-solid; the refinement makes the result exact.

Memory-bound target: per-core 16MB fp8 / ~400 GB/s ~= 40us stream; one
DMA per tile (both dim-chunks in a 3D AP) alternating across the two
HWDGE issue queues; TensorE ~27us of FWL weight loads + FD=1 matmuls
hides under the DMA stream.
"""

import sys

import numpy as np

for _p in ("/opt/trn_rl_repo",):
    if _p not in sys.path:
        sys.path.insert(0, _p)

import ml_dtypes

import concourse.bacc as bacc
import concourse.mybir as mybir
from concourse import tile
from concourse.bass_utils import run_bass_kernel_spmd

N_TRAIN = 500000
D = 256
CORES = 8
K = 3
P = 128
NS = N_TRAIN // CORES  # 62500 rows per core
BLOCKS = -(-NS // P)  # 489 psum columns
NSP = BLOCKS * P  # 62592 padded rows per core
BIG = 1.0e30
FP32 = mybir.dt.float32
F8 = mybir.dt.float8e4
U32 = mybir.dt.uint32
NP_F8 = ml_dtypes.float8_e4m3
SPLIT = 256  # epilogue half boundary (psum bank = 512 f32 max)


def build_knn(tc):
    """Per-core program: fp8 dot products via TensorE + top-8 epilogue."""
    nc = tc.nc
    x_ap = nc.dram_tensor("xq", [P, 2], F8, kind="ExternalInput").ap()
    a_ap = nc.dram_tensor("a", [2 * P, NSP], F8, kind="ExternalInput").ap()
    nrm_ap = nc.dram_tensor("nrm", [P, BLOCKS], FP32, kind="ExternalInput").ap()
    va_ap = nc.dram_tensor("out_vals_a", [P, 8], FP32, kind="ExternalOutput").ap()
    ia_ap = nc.dram_tensor("out_idx_a", [P, 8], U32, kind="ExternalOutput").ap()
    vb_ap = nc.dram_tensor("out_vals_b", [P, 8], FP32, kind="ExternalOutput").ap()
    ib_ap = nc.dram_tensor("out_idx_b", [P, 8], U32, kind="ExternalOutput").ap()

    with (
        tc.tile_pool(name="xp", bufs=1) as x_pool,
        tc.tile_pool(name="np", bufs=1) as n_pool,
        tc.tile_pool(name="inp", bufs=3) as in_pool,
        tc.tile_pool(name="psp", bufs=1, space="PSUM") as ps_pool,
        tc.tile_pool(name="outp", bufs=1) as out_pool,
    ):
        x_t = x_pool.tile([P, 2], F8)
        nc.sync.dma_start(out=x_t[:], in_=x_ap[:, :])
        nrm_t = n_pool.tile([P, BLOCKS], FP32)
        nc.sync.dma_start(out=nrm_t[:], in_=nrm_ap[:, :])

        # Two psum tiles so half A's epilogue overlaps half B's matmuls.
        ps_a = ps_pool.tile([P, SPLIT], FP32)
        ps_b = ps_pool.tile([P, BLOCKS - SPLIT], FP32)

        RT = 8192

        def epilogue_half(tag, lo, hi, ps):
            # compute only; output DMAs are deferred to the end so they
            # don't displace data DMAs mid-stream
            w = hi - lo
            m_t = out_pool.tile([P, w], FP32, tag=f"m_{tag}", name=f"m_{tag}")
            nc.vector.scalar_tensor_tensor(
                out=m_t[:],
                in0=ps[:],
                scalar=2.0,
                in1=nrm_t[:, lo:hi],
                op0=mybir.AluOpType.mult,
                op1=mybir.AluOpType.subtract,
            )
            valt = out_pool.tile([P, 8], FP32, tag=f"v_{tag}", name=f"v_{tag}")
            idxt = out_pool.tile([P, 8], U32, tag=f"i_{tag}", name=f"i_{tag}")
            nc.vector.max_with_indices(valt[:], idxt[:], m_t[:])
            return valt, idxt

        col = 0
        r = 0
        while r < NSP:
            rt = min(RT, NSP - r)
            t0 = in_pool.tile([P, RT], F8, tag="c0")
            t1 = in_pool.tile([P, RT], F8, tag="c1")
            nc.sync.dma_start(out=t0[:, 0:rt], in_=a_ap[0:P, r : r + rt])
            nc.sync.dma_start(out=t1[:, 0:rt], in_=a_ap[P : 2 * P, r : r + rt])
            for b in range(rt // P):
                if col < SPLIT:
                    pcol, ps = col, ps_a
                else:
                    pcol, ps = col - SPLIT, ps_b
                nc.tensor.matmul(
                    ps[:, pcol : pcol + 1],
                    lhsT=t0[:, b * P : (b + 1) * P],
                    rhs=x_t[:, 0:1],
                    start=True,
                    stop=False,
                )
                nc.tensor.matmul(
                    ps[:, pcol : pcol + 1],
                    lhsT=t1[:, b * P : (b + 1) * P],
                    rhs=x_t[:, 1:2],
                    start=False,
                    stop=True,
                )
                col += 1
                if col == SPLIT:
                    va_t, ia_t = epilogue_half("a", 0, SPLIT, ps_a)
            r += rt
        assert col == BLOCKS and r == NSP

        vb_t, ib_t = epilogue_half("b", SPLIT, BLOCKS, ps_b)
        nc.sync.dma_start(out=va_ap[:, :], in_=va_t[:])
        nc.scalar.dma_start(out=ia_ap[:, :], in_=ia_t[:])
        nc.sync.dma_start(out=vb_ap[:, :], in_=vb_t[:])
        nc.scalar.dma_start(out=ib_ap[:, :], in_=ib_t[:])


_PROGRAM_CACHE = {}


def get_program():
    if "nc" not in _PROGRAM_CACHE:
        nc = bacc.Bacc(
            "TRN2", target_bir_lowering=False, debug=False, num_devices=CORES
        )
        with tile.TileContext(nc) as tc:
            build_knn(tc)
        nc.compile()
        _PROGRAM_CACHE["nc"] = nc
    return _PROGRAM_CACHE["nc"]


def run_device(in_maps, trace=False, trace_cores=None):
    nc = get_program()
    return run_bass_kernel_spmd(
        nc, in_maps, list(range(CORES)), trace=trace, trace_cores=trace_cores
    )


def make_in_maps(x, train_data):
    x = np.asarray(x, dtype=np.float32)
    train_data = np.asarray(train_data, dtype=np.float32)
    td8 = train_data.astype(NP_F8)
    x8 = x.astype(NP_F8)
    xq = np.ascontiguousarray(x8.reshape(2, P).T)  # [128, 2]
    norms = np.einsum("nd,nd->n", train_data, train_data, dtype=np.float64)
    norms = norms.astype(np.float32)
    in_maps = []
    for c in range(CORES):
        a = np.zeros((2 * P, NSP), dtype=NP_F8)
        a[:, :NS] = td8[c * NS : (c + 1) * NS].T
        nrm = np.full(NSP, BIG, dtype=np.float32)
        nrm[:NS] = norms[c * NS : (c + 1) * NS]
        nrm = np.ascontiguousarray(nrm.reshape(BLOCKS, P).T)  # [128, BLOCKS]
        in_maps.append({"xq": xq, "a": a, "nrm": nrm})
    return in_maps


def merge_results(results, x, train_data, train_targets):
    """Merge per-core candidates; refine with exact distances on host."""
    x64 = np.asarray(x, dtype=np.float64)
    td = np.asarray(train_data)
    p_idx = np.arange(P, dtype=np.int64)[:, None]
    cands = []
    for c in range(len(results)):
        for vk, ik, off in (
            ("out_vals_a", "out_idx_a", 0),
            ("out_vals_b", "out_idx_b", SPLIT),
        ):
            v = np.asarray(results[c][vk], dtype=np.float64)
            ix = np.asarray(results[c][ik], dtype=np.int64) + off
            rl = ix * P + p_idx  # core-local row
            valid = (v > -BIG / 2) & (rl < NS)
            cands.append((c * NS + rl)[valid])
    g = np.unique(np.concatenate(cands))
    d2 = ((td[g].astype(np.float64) - x64) ** 2).sum(axis=1)
    order = np.lexsort((g, d2))  # distance asc, then index asc (top_k ties)
    top = g[order[:K]]
    knn_t = np.asarray(train_targets)[top]
    # torch .mode(): most frequent value, smallest value on ties
    counts = (knn_t[:, None] == knn_t[None, :]).sum(axis=1)
    sentinel = np.iinfo(knn_t.dtype).max
    cands_cls = np.where(counts == counts.max(), knn_t, sentinel)
    return cands_cls.min()


def kernel(x, train_data, train_targets):
    train_targets = np.asarray(train_targets)
    in_maps = make_in_maps(x, train_data)
    results = run_device(in_maps).results
    pred = merge_results(results, x, train_data, train_targets)
    return np.array(pred, dtype=train_targets.dtype)


# revision 23
# speedup vs baseline: 1.1795x; 1.0019x over previous
"""KNN classification kernel for Trainium2 (Bass/Tile), 8-core SPMD.

Problem: 1-query KNN over train_data [500000, 256] f32, K=3, 10 classes.
    distances = ||x - train_data||_2  -> top-3 smallest -> mode of targets.

Strategy (fp8 TensorE scan + exact host refinement):
  - Rank by m(t) = 2<x,t> - ||t||^2 (== -d^2 up to the constant ||x||^2).
  - Host precomputes exact fp32 row norms ||t||^2 and ships train_data
    TRANSPOSED in fp8 E4M3 (4x less HBM traffic than f32: 16MB/core).
  - Each core's TensorE computes <x,t> for its 62500 rows: the data tile
    is the STATIONARY operand (lhsT [128 dims x 128 rows], FWL-accelerated
    fp8 weight loads) and x is a 1-column moving operand; each 128-row
    block accumulates a [128,1] psum column over the two 128-dim chunks.
  - DVE epilogue (per column-half, so half A overlaps the matmul stream):
    m = 2*psum - norms, then max_with_indices -> per-partition top-8
    (value, block) candidates; DMA'd out.
  - Host merges 8 cores x 128 partitions x 16 candidates, recomputes
    EXACT fp64 distances for them from the original f32 data, takes the
    global top-3 by (distance, index) and the mode with smallest-on-tie.
    fp8 ranking error (std ~1.1) vs candidate margins (~70) makes the
    top-3 containment rock-solid; the refinement makes the result exact.

Memory-bound target: per-core 16MB fp8 / ~400 GB/s ~= 40us stream; one
DMA per tile (both dim-chunks in a 3D AP) alternating across the two
HWDGE issue queues; TensorE ~27us of FWL weight loads + FD=1 matmuls
hides under the DMA stream.
"""

import sys

import numpy as np

for _p in ("/opt/trn_rl_repo",):
    if _p not in sys.path:
        sys.path.insert(0, _p)

import ml_dtypes

import concourse.bacc as bacc
import concourse.mybir as mybir
from concourse import tile
from concourse.bass_utils import run_bass_kernel_spmd

N_TRAIN = 500000
D = 256
CORES = 8
K = 3
P = 128
NS = N_TRAIN // CORES  # 62500 rows per core
BLOCKS = -(-NS // P)  # 489 psum columns
NSP = BLOCKS * P  # 62592 padded rows per core
BIG = 1.0e30
FP32 = mybir.dt.float32
F8 = mybir.dt.float8e4
U32 = mybir.dt.uint32
NP_F8 = ml_dtypes.float8_e4m3
SPLIT = 256  # epilogue half boundary (psum bank = 512 f32 max)


def build_knn(tc):
    """Per-core program: fp8 dot products via TensorE + top-8 epilogue."""
    nc = tc.nc
    x_ap = nc.dram_tensor("xq", [P, 2], F8, kind="ExternalInput").ap()
    a_ap = nc.dram_tensor("a", [2 * P, NSP], F8, kind="ExternalInput").ap()
    nrm_ap = nc.dram_tensor("nrm", [P, BLOCKS], FP32, kind="ExternalInput").ap()
    va_ap = nc.dram_tensor("out_vals_a", [P, 8], FP32, kind="ExternalOutput").ap()
    ia_ap = nc.dram_tensor("out_idx_a", [P, 8], U32, kind="ExternalOutput").ap()
    vb_ap = nc.dram_tensor("out_vals_b", [P, 8], FP32, kind="ExternalOutput").ap()
    ib_ap = nc.dram_tensor("out_idx_b", [P, 8], U32, kind="ExternalOutput").ap()

    with (
        tc.tile_pool(name="xp", bufs=1) as x_pool,
        tc.tile_pool(name="np", bufs=1) as n_pool,
        tc.tile_pool(name="inp", bufs=5) as in_pool,
        tc.tile_pool(name="psp", bufs=1, space="PSUM") as ps_pool,
        tc.tile_pool(name="outp", bufs=1) as out_pool,
    ):
        x_t = x_pool.tile([P, 2], F8)
        nc.sync.dma_start(out=x_t[:], in_=x_ap[:, :])
        nrm_t = n_pool.tile([P, BLOCKS], FP32)
        nc.sync.dma_start(out=nrm_t[:], in_=nrm_ap[:, :])

        # Two psum tiles so half A's epilogue overlaps half B's matmuls.
        ps_a = ps_pool.tile([P, SPLIT], FP32)
        ps_b = ps_pool.tile([P, BLOCKS - SPLIT], FP32)

        sizes = [8192] * 7 + [4096, 1152]

        def epilogue_half(tag, lo, hi, ps):
            # compute only; output DMAs are deferred to the end so they
            # don't displace data DMAs mid-stream
            w = hi - lo
            m_t = out_pool.tile([P, w], FP32, tag=f"m_{tag}", name=f"m_{tag}")
            nc.vector.scalar_tensor_tensor(
                out=m_t[:],
                in0=ps[:],
                scalar=2.0,
                in1=nrm_t[:, lo:hi],
                op0=mybir.AluOpType.mult,
                op1=mybir.AluOpType.subtract,
            )
            valt = out_pool.tile([P, 8], FP32, tag=f"v_{tag}", name=f"v_{tag}")
            idxt = out_pool.tile([P, 8], U32, tag=f"i_{tag}", name=f"i_{tag}")
            nc.vector.max_with_indices(valt[:], idxt[:], m_t[:])
            return valt, idxt

        col = 0
        r = 0
        for rt in sizes:
            t0 = in_pool.tile([P, 8192], F8, tag="c0")
            t1 = in_pool.tile([P, 8192], F8, tag="c1")
            nc.sync.dma_start(out=t0[:, 0:rt], in_=a_ap[0:P, r : r + rt])
            nc.sync.dma_start(out=t1[:, 0:rt], in_=a_ap[P : 2 * P, r : r + rt])
            for b in range(rt // P):
                if col < SPLIT:
                    pcol, ps = col, ps_a
                else:
                    pcol, ps = col - SPLIT, ps_b
                nc.tensor.matmul(
                    ps[:, pcol : pcol + 1],
                    lhsT=t0[:, b * P : (b + 1) * P],
                    rhs=x_t[:, 0:1],
                    start=True,
                    stop=False,
                )
                nc.tensor.matmul(
                    ps[:, pcol : pcol + 1],
                    lhsT=t1[:, b * P : (b + 1) * P],
                    rhs=x_t[:, 1:2],
                    start=False,
                    stop=True,
                )
                col += 1
                if col == SPLIT:
                    va_t, ia_t = epilogue_half("a", 0, SPLIT, ps_a)
            r += rt
        assert col == BLOCKS and r == NSP

        vb_t, ib_t = epilogue_half("b", SPLIT, BLOCKS, ps_b)
        nc.sync.dma_start(out=va_ap[:, :], in_=va_t[:])
        nc.scalar.dma_start(out=ia_ap[:, :], in_=ia_t[:])
        nc.sync.dma_start(out=vb_ap[:, :], in_=vb_t[:])
        nc.scalar.dma_start(out=ib_ap[:, :], in_=ib_t[:])


_PROGRAM_CACHE = {}


def get_program():
    if "nc" not in _PROGRAM_CACHE:
        nc = bacc.Bacc(
            "TRN2", target_bir_lowering=False, debug=False, num_devices=CORES
        )
        with tile.TileContext(nc) as tc:
            build_knn(tc)
        nc.compile()
        _PROGRAM_CACHE["nc"] = nc
    return _PROGRAM_CACHE["nc"]


def run_device(in_maps, trace=False, trace_cores=None):
    nc = get_program()
    return run_bass_kernel_spmd(
        nc, in_maps, list(range(CORES)), trace=trace, trace_cores=trace_cores
    )


def make_in_maps(x, train_data):
    x = np.asarray(x, dtype=np.float32)
    train_data = np.asarray(train_data, dtype=np.float32)
    td8 = train_data.astype(NP_F8)
    x8 = x.astype(NP_F8)
    xq = np.ascontiguousarray(x8.reshape(2, P).T)  # [128, 2]
    norms = np.einsum("nd,nd->n", train_data, train_data, dtype=np.float64)
    norms = norms.astype(np.float32)
    in_maps = []
    for c in range(CORES):
        a = np.zeros((2 * P, NSP), dtype=NP_F8)
        a[:, :NS] = td8[c * NS : (c + 1) * NS].T
        nrm = np.full(NSP, BIG, dtype=np.float32)
        nrm[:NS] = norms[c * NS : (c + 1) * NS]
        nrm = np.ascontiguousarray(nrm.reshape(BLOCKS, P).T)  # [128, BLOCKS]
        in_maps.append({"xq": xq, "a": a, "nrm": nrm})
    return in_maps


def merge_results(results, x, train_data, train_targets):
    """Merge per-core candidates; refine with exact distances on host."""
    x64 = np.asarray(x, dtype=np.float64)
    td = np.asarray(train_data)
    p_idx = np.arange(P, dtype=np.int64)[:, None]
    cands = []
    for c in range(len(results)):
        for vk, ik, off in (
            ("out_vals_a", "out_idx_a", 0),
            ("out_vals_b", "out_idx_b", SPLIT),
        ):
            v = np.asarray(results[c][vk], dtype=np.float64)
            ix = np.asarray(results[c][ik], dtype=np.int64) + off
            rl = ix * P + p_idx  # core-local row
            valid = (v > -BIG / 2) & (rl < NS)
            cands.append((c * NS + rl)[valid])
    g = np.unique(np.concatenate(cands))
    d2 = ((td[g].astype(np.float64) - x64) ** 2).sum(axis=1)
    order = np.lexsort((g, d2))  # distance asc, then index asc (top_k ties)
    top = g[order[:K]]
    knn_t = np.asarray(train_targets)[top]
    # torch .mode(): most frequent value, smallest value on ties
    counts = (knn_t[:, None] == knn_t[None, :]).sum(axis=1)
    sentinel = np.iinfo(knn_t.dtype).max
    cands_cls = np.where(counts == counts.max(), knn_t, sentinel)
    return cands_cls.min()


def kernel(x, train_data, train_targets):
    train_targets = np.asarray(train_targets)
    in_maps = make_in_maps(x, train_data)
    results = run_device(in_maps).results
    pred = merge_results(results, x, train_data, train_targets)
    return np.array(pred, dtype=train_targets.dtype)
